# revision 7
# baseline (speedup 1.0000x reference)
"""CMAttention Trainium2 Bass kernel.

Reference computation (b=2, n=2048, dim=512, H=8 heads, dh=64, M=3 memory tokens):
    q = x @ wq;  k, v = split(x @ wkv);  per-head attention with 3 extra
    memory k/v tokens appended;  out = softmax(q k^T / 8) v;  y = out @ wo + bo.

Sharding: 16 (batch, head) pairs over 8 cores -> each core owns one batch and
two adjacent heads.  Per core everything is local; the out-projection is
row-sharded (per-head) and partial outputs are summed on the host (the
all-reduce of the sharding hint, done at gather time).

Device-side layout (per core, two heads "stacked" on partitions 0-63 / 64-127):
    xT   [4][128, 2048]   x[b]^T, contraction dim c on partitions (host pre-transposed)
    qT   [128, 2048]      q^T = wq_s^T-chunks @ xT    (d_global on partitions)
    kT   [128, 2052]      k^T * 1/8 (scale folded into wk on host) ++ memory keys
    v    via PE transpose -> v_aug[h] [128, 17*65]: per j-tile [128, 64+1(ones)]
    scores^T s[j, i] = kT_h^T-slice.T @ qT_h  -> PSUM [128(j), 1024(i)]
    exp on ScalarE PSUM->SBUF
    av:  out_h^T[65, i] += v_aug_jt.T @ exp_jt   (row 64 = softmax denominator)
    out-projection per head + per-partition reciprocal scaling, host sums partials.
"""

import sys

for _p in ("/opt/trn_rl_repo", "/root/.axon_site/_ro/trn_rl_repo"):
    if _p not in sys.path:
        sys.path.insert(0, _p)

import numpy as np

import concourse.bacc as bacc
import concourse.mybir as mybir
import concourse.tile as tile
from concourse import bass_utils
from concourse.masks import make_identity

F32 = mybir.dt.float32
FR = mybir.dt.float32r  # fp32 bits, single-pass PE matmul (1 cycle/row for N>=256)
AF = mybir.ActivationFunctionType
ALU = mybir.AluOpType

H, DH, M = 8, 64, 3
DIM = 512
INNER = H * DH
NSEQ = 2048
B = 2
N_CORES = 8
SCALE = DH ** -0.5
SQRT_M = float(np.sqrt(M))

_CACHE = {}


def _emit(nc, tc, n):
    """Emit the per-core program. n = sequence length (queries)."""
    n_it = n // 128          # i-tiles of 128 queries
    n_ic = n // 512          # 512-query column chunks
    n_ih = n // 1024         # 1024-query halves for the attention loop
    n_jt = n // 128 + 1      # j-tiles: n/128 full + 1 memory tile (3 rows)
    VA = 65                  # v_aug cols per j-tile: 64 dims + ones column

    ap_xt = nc.dram_tensor("xt", [4, 128, n], FR, kind="ExternalInput").ap()
    ap_wq = nc.dram_tensor("wq_s", [4, 128, 128], FR, kind="ExternalInput").ap()
    ap_wk = nc.dram_tensor("wk_s", [4, 128, 128], FR, kind="ExternalInput").ap()
    ap_wv = nc.dram_tensor("wv_s", [4, 128, 128], FR, kind="ExternalInput").ap()
    ap_wo = nc.dram_tensor("wo_s", [128, DIM], FR, kind="ExternalInput").ap()
    ap_mkT = nc.dram_tensor("mkT_s", [128, M], FR, kind="ExternalInput").ap()
    ap_mv = nc.dram_tensor("mv_s", [M, 128], FR, kind="ExternalInput").ap()
    ap_out = nc.dram_tensor("out", [n_it, 128, DIM], F32, kind="ExternalOutput").ap()

    with (
        tc.tile_pool(name="persist", bufs=1) as per,
        tc.tile_pool(name="dram", bufs=1, space="DRAM") as dpool,
    ):
        xt = [per.tile([128, n], FR, tag=f"xt{c}", name=f"xt{c}") for c in range(4)]
        wq_sb = [per.tile([128, 128], FR, tag=f"wq{c}", name=f"wq{c}") for c in range(4)]
        wk_sb = [per.tile([128, 128], FR, tag=f"wk{c}", name=f"wk{c}") for c in range(4)]
        wv_sb = [per.tile([128, 128], FR, tag=f"wv{c}", name=f"wv{c}") for c in range(4)]
        wo_sb = per.tile([128, DIM], FR, tag="wo", name="wo")
        qT = per.tile([128, n], FR, tag="qT", name="qT")
        kT = per.tile([128, n + 4], FR, tag="kT", name="kT")
        vT = per.tile([128, n], F32, tag="vT", name="vT")
        v_aug = [per.tile([128, n_jt * VA], FR, tag=f"vaug{h}", name=f"vaug{h}") for h in range(2)]
        oT = per.tile([128, n], FR, tag="oT", name="oT")
        rec_col = [per.tile([128, n_it], F32, tag=f"rec{h}", name=f"rec{h}") for h in range(2)]
        ident = per.tile([128, 128], F32, tag="ident", name="ident")

        nc.sync.dma_start(out=wo_sb, in_=ap_wo)
        nc.sync.dma_start(out=kT[:, n : n + M], in_=ap_mkT)
        make_identity(nc, ident[:])
        for h in range(2):
            nc.vector.memset(v_aug[h][:].bitcast(F32), 1.0)
            nc.sync.dma_start(
                out=v_aug[h][0:M, (n_jt - 1) * VA : (n_jt - 1) * VA + 64],
                in_=ap_mv[:, h * 64 : (h + 1) * 64],
            )

        # ---- projections, DMA interleaved per c-chunk so PE starts early
        with tc.tile_pool(name="proj_ps", bufs=8, space="PSUM") as proj_ps:
            q_ps = [proj_ps.tile([128, 512], F32, tag="proj", name="qps") for _ in range(n_ic)]
            k_ps = [proj_ps.tile([128, 512], F32, tag="proj", name="kps") for _ in range(n_ic)]
            for c in range(4):
                nc.sync.dma_start(out=xt[c], in_=ap_xt[c])
                nc.sync.dma_start(out=wq_sb[c], in_=ap_wq[c])
                nc.sync.dma_start(out=wk_sb[c], in_=ap_wk[c])
                nc.sync.dma_start(out=wv_sb[c], in_=ap_wv[c])
                for ic in range(n_ic):
                    nc.tensor.matmul(
                        q_ps[ic][:],
                        wq_sb[c][:],
                        xt[c][:, ic * 512 : (ic + 1) * 512],
                        start=(c == 0),
                        stop=(c == 3),
                    )
                    nc.tensor.matmul(
                        k_ps[ic][:],
                        wk_sb[c][:],
                        xt[c][:, ic * 512 : (ic + 1) * 512],
                        start=(c == 0),
                        stop=(c == 3),
                    )
            for ic in range(n_ic):
                nc.scalar.copy(out=qT[:, ic * 512 : (ic + 1) * 512], in_=q_ps[ic][:])
                nc.scalar.copy(out=kT[:, ic * 512 : (ic + 1) * 512], in_=k_ps[ic][:])
            v_ps = [proj_ps.tile([128, 512], F32, tag="proj", name="vps") for _ in range(n_ic)]
            for c in range(4):
                for ic in range(n_ic):
                    nc.tensor.matmul(
                        v_ps[ic][:],
                        wv_sb[c][:],
                        xt[c][:, ic * 512 : (ic + 1) * 512],
                        start=(c == 0),
                        stop=(c == 3),
                    )
            for ic in range(n_ic):
                nc.scalar.copy(out=vT[:, ic * 512 : (ic + 1) * 512], in_=v_ps[ic][:])

        # v_aug: transpose vT 128x128 blocks into per-head [j, d] + ones col
        with tc.tile_pool(name="tr_ps", bufs=2, space="PSUM") as tr_ps:
            for jt in range(n_jt - 1):
                pt = tr_ps.tile([128, 128], F32, tag="tr", name="tr")
                nc.tensor.transpose(
                    pt[:], vT[:, jt * 128 : (jt + 1) * 128], ident[:]
                )
                for h in range(2):
                    nc.vector.tensor_copy(
                        out=v_aug[h][:, jt * VA : jt * VA + 64],
                        in_=pt[:, h * 64 : (h + 1) * 64],
                    )

        # ---- attention, heads sequential (PSUM: scores 2x2 banks + av 4 banks)
        with (
            tc.tile_pool(name="s_ps", bufs=2, space="PSUM") as s_ps_pool,
            tc.tile_pool(name="av_ps", bufs=1, space="PSUM") as av_ps_pool,
            tc.tile_pool(name="exp_sb", bufs=3) as exp_pool,
            tc.tile_pool(name="small", bufs=2) as small,
        ):
            for h in range(2):
                hp = h * 64
                av = av_ps_pool.tile([VA, n], F32, tag="av", name="av")
                for jt in range(n_jt):
                    kj = 128 if jt < n_jt - 1 else M
                    lhs_k = kT[hp : hp + 64, jt * 128 : jt * 128 + kj]
                    va = v_aug[h][0:kj, jt * VA : (jt + 1) * VA]
                    for ih in range(n_ih):
                        sp = s_ps_pool.tile([128, 1024], F32, tag="sp", name="sp")
                        for sub in range(2):
                            o0 = sub * 512
                            nc.tensor.matmul(
                                sp[0:kj, o0 : o0 + 512],
                                lhs_k,
                                qT[hp : hp + 64, ih * 1024 + o0 : ih * 1024 + o0 + 512],
                                start=True,
                                stop=True,
                            )
                        et = exp_pool.tile([128, 1024], FR, tag="exp", name="exp")
                        nc.scalar.activation(out=et[0:kj, :], in_=sp[0:kj, :], func=AF.Exp)
                        for sub in range(2):
                            o0 = sub * 512
                            nc.tensor.matmul(
                                av[:, ih * 1024 + o0 : ih * 1024 + o0 + 512],
                                va,
                                et[0:kj, o0 : o0 + 512],
                                start=(jt == 0),
                                stop=(jt == n_jt - 1),
                            )
                # unnormalized head output + denominators
                nc.vector.tensor_copy(out=oT[hp : hp + 64, :], in_=av[0:64, :])
                den = small.tile([1, n], F32, tag="den", name="den")
                nc.vector.tensor_copy(out=den[:], in_=av[64:65, :])
                rd = dpool.tile([n], F32, tag=f"rd{h}", name=f"rd{h}")
                nc.sync.dma_start(out=rd[:], in_=den[:])
                den_col = small.tile([128, n_it], F32, tag="den_col", name="den_col")
                nc.sync.dma_start(
                    out=den_col[:], in_=rd[:].rearrange("(t p) -> p t", p=128)
                )
                nc.vector.reciprocal(out=rec_col[h][:], in_=den_col[:])

        # ---- out-projection per head (row-packed K=64) + normalize + store
        with (
            tc.tile_pool(name="op_ps", bufs=4, space="PSUM") as op_ps,
            tc.tile_pool(name="ostage", bufs=3) as ostage,
        ):
            for t in range(n_it):
                p0 = op_ps.tile([128, 512], F32, tag="op", name="op")
                p1 = op_ps.tile([128, 512], F32, tag="op", name="op")
                nc.tensor.matmul(
                    p0[:],
                    oT[0:64, t * 128 : (t + 1) * 128],
                    wo_sb[0:64, :],
                    start=True,
                    stop=True,
                )
                nc.tensor.matmul(
                    p1[:],
                    oT[64:128, t * 128 : (t + 1) * 128],
                    wo_sb[64:128, :],
                    start=True,
                    stop=True,
                )
                a0 = ostage.tile([128, 512], F32, tag="tmp", name="a0")
                nc.scalar.mul(a0[:], p0[:], rec_col[0][:, t : t + 1])
                outb = ostage.tile([128, 512], F32, tag="outb", name="outb")
                nc.vector.scalar_tensor_tensor(
                    out=outb[:],
                    in0=p1[:],
                    scalar=rec_col[1][:, t : t + 1],
                    in1=a0[:],
                    op0=ALU.mult,
                    op1=ALU.add,
                )
                nc.sync.dma_start(out=ap_out[t], in_=outb[:])


def _build(n=NSEQ):
    if n in _CACHE:
        return _CACHE[n]
    nc = bacc.Bacc("TRN2", debug=False, num_devices=N_CORES)
    with tile.TileContext(nc) as tc:
        _emit(nc, tc, n)
    nc.compile()
    _CACHE[n] = nc
    return nc


def _prep_in_maps(x, wq, wkv, wo, m_k, m_v, n):
    x = np.asarray(x, np.float32)
    wq = np.asarray(wq, np.float32)
    wkv = np.asarray(wkv, np.float32)
    wo = np.asarray(wo, np.float32)
    m_k = np.asarray(m_k, np.float32)
    m_v = np.asarray(m_v, np.float32)

    wk = wkv[:, :INNER]
    wv = wkv[:, INNER:]
    # memory tokens: flat reshape (M, INNER) -> (H, M, DH), exactly as reference
    mk_heads = m_k.reshape(M * INNER).reshape(H, M, DH)  # * SQRT_DH * SCALE == 1.0
    mv_heads = m_v.reshape(M * INNER).reshape(H, M, DH) * SQRT_M

    in_maps = []
    for cid in range(N_CORES):
        b = cid // 4
        h0 = 2 * (cid % 4)
        sl = slice(h0 * DH, (h0 + 2) * DH)
        in_maps.append(
            {
                "xt": np.ascontiguousarray(x[b].T).reshape(4, 128, n),
                "wq_s": np.ascontiguousarray(wq[:, sl]).reshape(4, 128, 128),
                "wk_s": np.ascontiguousarray(wk[:, sl] * SCALE).reshape(4, 128, 128),
                "wv_s": np.ascontiguousarray(wv[:, sl]).reshape(4, 128, 128),
                "wo_s": np.ascontiguousarray(wo[sl, :]),
                "mkT_s": np.ascontiguousarray(
                    np.concatenate([mk_heads[h0].T, mk_heads[h0 + 1].T], axis=0)
                ),
                "mv_s": np.ascontiguousarray(
                    np.concatenate([mv_heads[h0], mv_heads[h0 + 1]], axis=1)
                ),
            }
        )
    return in_maps


def _gather(results, bo, n):
    bo = np.asarray(bo, np.float32)
    out = np.zeros((B, n, DIM), np.float32)
    for cid in range(N_CORES):
        out[cid // 4] += results[cid]["out"].reshape(n, DIM)
    out += bo
    return out


def run(x, wq, wkv, wo, bo, m_k, m_v, trace=False, n=NSEQ):
    nc = _build(n)
    in_maps = _prep_in_maps(x, wq, wkv, wo, m_k, m_v, n)
    res = bass_utils.run_bass_kernel_spmd(
        nc, in_maps, core_ids=list(range(N_CORES)), trace=trace
    )
    return _gather(res.results, bo, n), res


def kernel(x, wq, wkv, wo, bo, m_k, m_v):
    out, _ = run(x, wq, wkv, wo, bo, m_k, m_v)
    return out


# revision 8
# speedup vs baseline: 1.0745x; 1.0745x over previous
"""CMAttention Trainium2 Bass kernel.

Reference computation (b=2, n=2048, dim=512, H=8 heads, dh=64, M=3 memory tokens):
    q = x @ wq;  k, v = split(x @ wkv);  per-head attention with 3 extra
    memory k/v tokens appended;  out = softmax(q k^T / 8) v;  y = out @ wo + bo.

Sharding: 16 (batch, head) pairs over 8 cores -> each core owns one batch and
two adjacent heads.  Per core everything is local; the out-projection is
row-sharded (per-head) and partial outputs are summed on the host (the
all-reduce of the sharding hint, done at gather time).

Device-side layout (per core, two heads "stacked" on partitions 0-63 / 64-127):
    xT   [4][128, 2048]   x[b]^T, contraction dim c on partitions (host pre-transposed)
    qT   [128, 2048]      q^T = wq_s^T-chunks @ xT    (d_global on partitions)
    kT   [128, 2052]      k^T * 1/8 (scale folded into wk on host) ++ memory keys
    v    via PE transpose -> v_aug[h] [128, 17*65]: per j-tile [128, 64+1(ones)]
    scores^T s[j, i] = kT_h^T-slice.T @ qT_h  -> PSUM [128(j), 1024(i)]
    exp on ScalarE PSUM->SBUF
    av:  out_h^T[65, i] += v_aug_jt.T @ exp_jt   (row 64 = softmax denominator)
    out-projection per head + per-partition reciprocal scaling, host sums partials.
"""

import sys

for _p in ("/opt/trn_rl_repo", "/root/.axon_site/_ro/trn_rl_repo"):
    if _p not in sys.path:
        sys.path.insert(0, _p)

import ml_dtypes
import numpy as np

import concourse.bacc as bacc
import concourse.mybir as mybir
import concourse.tile as tile
from concourse import bass_utils
from concourse.masks import make_identity

F32 = mybir.dt.float32
FR = mybir.dt.float32r  # fp32 bits, single-pass PE matmul (1 cycle/row for N>=256)
BF = mybir.dt.bfloat16
AF = mybir.ActivationFunctionType
ALU = mybir.AluOpType

H, DH, M = 8, 64, 3
DIM = 512
INNER = H * DH
NSEQ = 2048
B = 2
N_CORES = 8
SCALE = DH ** -0.5
SQRT_M = float(np.sqrt(M))

_CACHE = {}


def _emit(nc, tc, n):
    """Emit the per-core program. n = sequence length (queries)."""
    n_it = n // 128          # i-tiles of 128 queries
    n_ic = n // 512          # 512-query column chunks
    n_ih = n // 1024         # 1024-query halves for the attention loop
    n_jt = n // 128 + 1      # j-tiles: n/128 full + 1 memory tile (3 rows)
    VA = 65                  # v_aug cols per j-tile: 64 dims + ones column

    ap_xt = nc.dram_tensor("xt", [4, 128, n], FR, kind="ExternalInput").ap()
    ap_wq = nc.dram_tensor("wq_s", [4, 128, 128], FR, kind="ExternalInput").ap()
    ap_wk = nc.dram_tensor("wk_s", [4, 128, 128], FR, kind="ExternalInput").ap()
    ap_wv = nc.dram_tensor("wv_s", [4, 128, 128], FR, kind="ExternalInput").ap()
    ap_wo = nc.dram_tensor("wo_s", [128, DIM], FR, kind="ExternalInput").ap()
    ap_mkT = nc.dram_tensor("mkT_s", [128, M], BF, kind="ExternalInput").ap()
    ap_mv = nc.dram_tensor("mv_s", [M, 128], BF, kind="ExternalInput").ap()
    ap_out = nc.dram_tensor("out", [n_it, 128, DIM], F32, kind="ExternalOutput").ap()

    with (
        tc.tile_pool(name="persist", bufs=1) as per,
        tc.tile_pool(name="dram", bufs=1, space="DRAM") as dpool,
    ):
        xt = [per.tile([128, n], FR, tag=f"xt{c}", name=f"xt{c}") for c in range(4)]
        wq_sb = [per.tile([128, 128], FR, tag=f"wq{c}", name=f"wq{c}") for c in range(4)]
        wk_sb = [per.tile([128, 128], FR, tag=f"wk{c}", name=f"wk{c}") for c in range(4)]
        wv_sb = [per.tile([128, 128], FR, tag=f"wv{c}", name=f"wv{c}") for c in range(4)]
        wo_sb = per.tile([128, DIM], FR, tag="wo", name="wo")
        qT = per.tile([128, n], BF, tag="qT", name="qT")
        kT = per.tile([128, n + 4], BF, tag="kT", name="kT")
        vT = per.tile([128, n], F32, tag="vT", name="vT")
        v_aug = [per.tile([128, n_jt * VA], BF, tag=f"vaug{h}", name=f"vaug{h}") for h in range(2)]
        oT = per.tile([128, n], FR, tag="oT", name="oT")
        rec_col = [per.tile([128, n_it], F32, tag=f"rec{h}", name=f"rec{h}") for h in range(2)]
        ident = per.tile([128, 128], F32, tag="ident", name="ident")

        nc.sync.dma_start(out=wo_sb, in_=ap_wo)
        nc.sync.dma_start(out=kT[:, n : n + M], in_=ap_mkT)
        make_identity(nc, ident[:])
        for h in range(2):
            nc.vector.memset(v_aug[h][:], 1.0)
            nc.sync.dma_start(
                out=v_aug[h][0:M, (n_jt - 1) * VA : (n_jt - 1) * VA + 64],
                in_=ap_mv[:, h * 64 : (h + 1) * 64],
            )

        # ---- projections, DMA interleaved per c-chunk so PE starts early
        with tc.tile_pool(name="proj_ps", bufs=8, space="PSUM") as proj_ps:
            q_ps = [proj_ps.tile([128, 512], F32, tag="proj", name="qps") for _ in range(n_ic)]
            k_ps = [proj_ps.tile([128, 512], F32, tag="proj", name="kps") for _ in range(n_ic)]
            for c in range(4):
                nc.sync.dma_start(out=xt[c], in_=ap_xt[c])
                nc.sync.dma_start(out=wq_sb[c], in_=ap_wq[c])
                nc.sync.dma_start(out=wk_sb[c], in_=ap_wk[c])
                nc.sync.dma_start(out=wv_sb[c], in_=ap_wv[c])
                for ic in range(n_ic):
                    nc.tensor.matmul(
                        q_ps[ic][:],
                        wq_sb[c][:],
                        xt[c][:, ic * 512 : (ic + 1) * 512],
                        start=(c == 0),
                        stop=(c == 3),
                    )
                    nc.tensor.matmul(
                        k_ps[ic][:],
                        wk_sb[c][:],
                        xt[c][:, ic * 512 : (ic + 1) * 512],
                        start=(c == 0),
                        stop=(c == 3),
                    )
            for ic in range(n_ic):
                nc.scalar.copy(out=qT[:, ic * 512 : (ic + 1) * 512], in_=q_ps[ic][:])
                nc.scalar.copy(out=kT[:, ic * 512 : (ic + 1) * 512], in_=k_ps[ic][:])
            v_ps = [proj_ps.tile([128, 512], F32, tag="proj", name="vps") for _ in range(n_ic)]
            for c in range(4):
                for ic in range(n_ic):
                    nc.tensor.matmul(
                        v_ps[ic][:],
                        wv_sb[c][:],
                        xt[c][:, ic * 512 : (ic + 1) * 512],
                        start=(c == 0),
                        stop=(c == 3),
                    )
            for ic in range(n_ic):
                nc.scalar.copy(out=vT[:, ic * 512 : (ic + 1) * 512], in_=v_ps[ic][:])

        # v_aug: transpose vT 128x128 blocks into per-head [j, d] + ones col
        with tc.tile_pool(name="tr_ps", bufs=2, space="PSUM") as tr_ps:
            for jt in range(n_jt - 1):
                pt = tr_ps.tile([128, 128], F32, tag="tr", name="tr")
                nc.tensor.transpose(
                    pt[:], vT[:, jt * 128 : (jt + 1) * 128], ident[:]
                )
                for h in range(2):
                    nc.vector.tensor_copy(
                        out=v_aug[h][:, jt * VA : jt * VA + 64],
                        in_=pt[:, h * 64 : (h + 1) * 64],
                    )

        # ---- attention, heads sequential (PSUM: scores 2x2 banks + av 4 banks)
        with (
            tc.tile_pool(name="s_ps", bufs=2, space="PSUM") as s_ps_pool,
            tc.tile_pool(name="av_ps", bufs=1, space="PSUM") as av_ps_pool,
            tc.tile_pool(name="exp_sb", bufs=3) as exp_pool,
            tc.tile_pool(name="small", bufs=2) as small,
        ):
            for h in range(2):
                hp = h * 64
                av = av_ps_pool.tile([VA, n], F32, tag="av", name="av")
                for jt in range(n_jt):
                    kj = 128 if jt < n_jt - 1 else M
                    lhs_k = kT[hp : hp + 64, jt * 128 : jt * 128 + kj]
                    va = v_aug[h][0:kj, jt * VA : (jt + 1) * VA]
                    for ih in range(n_ih):
                        sp = s_ps_pool.tile([128, 1024], F32, tag="sp", name="sp")
                        for sub in range(2):
                            o0 = sub * 512
                            nc.tensor.matmul(
                                sp[0:kj, o0 : o0 + 512],
                                lhs_k,
                                qT[hp : hp + 64, ih * 1024 + o0 : ih * 1024 + o0 + 512],
                                start=True,
                                stop=True,
                            )
                        et = exp_pool.tile([128, 1024], BF, tag="exp", name="exp")
                        nc.scalar.activation(out=et[0:kj, :], in_=sp[0:kj, :], func=AF.Exp)
                        for sub in range(2):
                            o0 = sub * 512
                            nc.tensor.matmul(
                                av[:, ih * 1024 + o0 : ih * 1024 + o0 + 512],
                                va,
                                et[0:kj, o0 : o0 + 512],
                                start=(jt == 0),
                                stop=(jt == n_jt - 1),
                            )
                # unnormalized head output + denominators
                nc.vector.tensor_copy(out=oT[hp : hp + 64, :], in_=av[0:64, :])
                den = small.tile([1, n], F32, tag="den", name="den")
                nc.vector.tensor_copy(out=den[:], in_=av[64:65, :])
                rd = dpool.tile([n], F32, tag=f"rd{h}", name=f"rd{h}")
                nc.sync.dma_start(out=rd[:], in_=den[:])
                den_col = small.tile([128, n_it], F32, tag="den_col", name="den_col")
                nc.sync.dma_start(
                    out=den_col[:], in_=rd[:].rearrange("(t p) -> p t", p=128)
                )
                nc.vector.reciprocal(out=rec_col[h][:], in_=den_col[:])

        # ---- out-projection per head (row-packed K=64) + normalize + store
        with (
            tc.tile_pool(name="op_ps", bufs=4, space="PSUM") as op_ps,
            tc.tile_pool(name="ostage", bufs=3) as ostage,
        ):
            for t in range(n_it):
                p0 = op_ps.tile([128, 512], F32, tag="op", name="op")
                p1 = op_ps.tile([128, 512], F32, tag="op", name="op")
                nc.tensor.matmul(
                    p0[:],
                    oT[0:64, t * 128 : (t + 1) * 128],
                    wo_sb[0:64, :],
                    start=True,
                    stop=True,
                )
                nc.tensor.matmul(
                    p1[:],
                    oT[64:128, t * 128 : (t + 1) * 128],
                    wo_sb[64:128, :],
                    start=True,
                    stop=True,
                )
                a0 = ostage.tile([128, 512], F32, tag="tmp", name="a0")
                nc.scalar.mul(a0[:], p0[:], rec_col[0][:, t : t + 1])
                outb = ostage.tile([128, 512], F32, tag="outb", name="outb")
                nc.vector.scalar_tensor_tensor(
                    out=outb[:],
                    in0=p1[:],
                    scalar=rec_col[1][:, t : t + 1],
                    in1=a0[:],
                    op0=ALU.mult,
                    op1=ALU.add,
                )
                nc.sync.dma_start(out=ap_out[t], in_=outb[:])


def _build(n=NSEQ):
    if n in _CACHE:
        return _CACHE[n]
    nc = bacc.Bacc("TRN2", debug=False, num_devices=N_CORES)
    with tile.TileContext(nc) as tc:
        _emit(nc, tc, n)
    nc.compile()
    _CACHE[n] = nc
    return nc


def _prep_in_maps(x, wq, wkv, wo, m_k, m_v, n):
    x = np.asarray(x, np.float32)
    wq = np.asarray(wq, np.float32)
    wkv = np.asarray(wkv, np.float32)
    wo = np.asarray(wo, np.float32)
    m_k = np.asarray(m_k, np.float32)
    m_v = np.asarray(m_v, np.float32)

    wk = wkv[:, :INNER]
    wv = wkv[:, INNER:]
    # memory tokens: flat reshape (M, INNER) -> (H, M, DH), exactly as reference
    mk_heads = m_k.reshape(M * INNER).reshape(H, M, DH)  # * SQRT_DH * SCALE == 1.0
    mv_heads = m_v.reshape(M * INNER).reshape(H, M, DH) * SQRT_M

    in_maps = []
    for cid in range(N_CORES):
        b = cid // 4
        h0 = 2 * (cid % 4)
        sl = slice(h0 * DH, (h0 + 2) * DH)
        in_maps.append(
            {
                "xt": np.ascontiguousarray(x[b].T).reshape(4, 128, n),
                "wq_s": np.ascontiguousarray(wq[:, sl]).reshape(4, 128, 128),
                "wk_s": np.ascontiguousarray(wk[:, sl] * SCALE).reshape(4, 128, 128),
                "wv_s": np.ascontiguousarray(wv[:, sl]).reshape(4, 128, 128),
                "wo_s": np.ascontiguousarray(wo[sl, :]),
                "mkT_s": np.ascontiguousarray(
                    np.concatenate([mk_heads[h0].T, mk_heads[h0 + 1].T], axis=0)
                ).astype(ml_dtypes.bfloat16),
                "mv_s": np.ascontiguousarray(
                    np.concatenate([mv_heads[h0], mv_heads[h0 + 1]], axis=1)
                ).astype(ml_dtypes.bfloat16),
            }
        )
    return in_maps


def _gather(results, bo, n):
    bo = np.asarray(bo, np.float32)
    out = np.zeros((B, n, DIM), np.float32)
    for cid in range(N_CORES):
        out[cid // 4] += results[cid]["out"].reshape(n, DIM)
    out += bo
    return out


def run(x, wq, wkv, wo, bo, m_k, m_v, trace=False, n=NSEQ):
    nc = _build(n)
    in_maps = _prep_in_maps(x, wq, wkv, wo, m_k, m_v, n)
    res = bass_utils.run_bass_kernel_spmd(
        nc, in_maps, core_ids=list(range(N_CORES)), trace=trace
    )
    return _gather(res.results, bo, n), res


def kernel(x, wq, wkv, wo, bo, m_k, m_v):
    out, _ = run(x, wq, wkv, wo, bo, m_k, m_v)
    return out
